# revision 1
# baseline (speedup 1.0000x reference)
"""Trainium2 Bass kernel for nn_AttentionPropagation.

Shapes (hardcoded): B=4, C=128, H=4 heads, D=32, N=2048.
Sharding: 8 cores = (batch b) x (sequence half). The network is pointwise in
the query position n everywhere except K/V, so each core takes x1[b,:,half]
(1024 query positions) plus the full x2[b] (keys/values) and produces
out[b,:,half] with no cross-core communication. K/V work is replicated
across the 2 cores sharing a batch (cheap: ~2% of FLOPs).

Math folding done host-side (exact):
 - 1/sqrt(D) folded into wq/bq.
 - bk dropped: adds a per-query constant to scores -> cancels in softmax.
 - bv folded into the mh-projection bias: softmax rows sum to 1, so
   score @ (v + bv) = score @ v + bv, and wm @ (av + bv) + bm = wm @ av + (bm + wm@bv).
 - BatchNorm (inference) folded into wc1/bias.
 - kv_mask is all ones per the spec (fill=ones) -> no-op, ignored.

Device kernel per core (all matmuls in bf16, fp32 PSUM accumulate):
 - Q = wq'@x1s + bq'   [128, 1024]
 - K = wk@x2           [128, 2048]
 - VT = x2^T wv^T computed directly transposed via matmul, stored per
   (m-block, head) as lhsT tiles [128, 64] = [V^T block | ones32].
 - scores^T[m-block, n-chunk] per head via standard base-0 matmuls
   (K/Q kept head-major at partitions 0..32) -> psum [128, 2 heads * 512].
 - one Exp ACTIVATE per wave over [128, 1024] psum -> expT sbuf.
 - av += [V^T|ones].T @ expT accumulated over 16 m-blocks into psum bank h;
   rows 0-31 = unnormalized head output, rows 32-63 = sum(exp) replicated.
 - normalize: rec = 1/sums (DVE exact reciprocal), av_all = av * rec.
 - tail: mh -> concat -> c1 -> (folded BN) relu -> c2 -> + x1s.
"""

import os
import sys

import numpy as np

sys.path.insert(0, "/opt/trn_rl_repo")

_CACHE = {}

P = 128
B, C, H, D, N = 4, 128, 4, 32, 2048
NH = N // 2  # per-core query positions


def _build_nc():
    import concourse.bass as bass
    import concourse.mybir as mybir
    import concourse.tile as tile
    from concourse import bacc
    from concourse.bass import ts

    f32 = mybir.dt.float32
    bf16 = mybir.dt.bfloat16
    AF = mybir.ActivationFunctionType
    OP = mybir.AluOpType

    nc = bacc.Bacc()
    x1s = nc.declare_dram_parameter("x1s", [P, NH], f32, isOutput=False)
    x2b = nc.declare_dram_parameter("x2b", [P, N], f32, isOutput=False)
    # all weights packed into one tensor (cols: wqT 0:128, wkT 128:256,
    # wvT 256:384, wmT 384:512, wc1T 512:1024 (k*256+o), wc2T 1024:1280)
    wpack = nc.declare_dram_parameter("wpack", [P, 1280], f32, isOutput=False)
    # biases packed (cols: bq4 0:4 [rows 0:32], bm 4, b1 5:7, bc2 7)
    bpack = nc.declare_dram_parameter("bpack", [P, 8], f32, isOutput=False)
    out_d = nc.declare_dram_parameter("out", [P, NH], f32, isOutput=True)

    with tile.TileContext(nc) as tc:
        with (
            tc.tile_pool(name="consts", bufs=1) as consts,
            tc.tile_pool(name="main", bufs=1) as main,
            tc.tile_pool(name="work", bufs=3) as work,
            tc.tile_pool(name="recp", bufs=2) as recp,
        ):
            # ---- load inputs (fp32) and round matmul operands to bf16 ----
            x1t = main.tile([P, NH], f32)
            nc.sync.dma_start(x1t[:], x1s[:])
            x1r = main.tile([P, NH], bf16)
            nc.vector.tensor_copy(x1r[:], x1t[:])

            x2stg = main.tile([P, N], f32)
            nc.sync.dma_start(x2stg[:], x2b[:])
            x2r = main.tile([P, N], bf16)
            nc.vector.tensor_copy(x2r[:], x2stg[:])

            wstg = consts.tile([P, 1280], f32)
            nc.sync.dma_start(wstg[:], wpack[:])
            wr = consts.tile([P, 1280], bf16)
            nc.vector.tensor_copy(wr[:], wstg[:])
            wq_t = wr[:, 0:128]
            wk_t = wr[:, 128:256]
            wv_t = wr[:, 256:384]
            wm_t = wr[:, 384:512]

            def wc1_l(k, oh):  # lhsT chunk [128 in, 128 out]
                return wr[:, 512 + k * 256 + oh * 128 : 512 + k * 256 + oh * 128 + 128]

            def wc2_l(oh):
                return wr[:, 1024 + oh * 128 : 1024 + oh * 128 + 128]

            bp_t = consts.tile([P, 8], f32)
            nc.sync.dma_start(bp_t[:], bpack[:])
            bq_t = bp_t[:, 0:1]  # bq*s, per channel
            bm_t = bp_t[:, 4:5]
            b1_t = bp_t[:, 5:7]
            bc2_t = bp_t[:, 7:8]

            # head-major Q/K at partition base 0, zero-padded to 128 partitions
            # (sub-128-contract matmuls hang this stack; zero rows make every
            # attention matmul a standard 128-contract matmul).
            Q4 = main.tile([P, H, NH], bf16)
            K4 = main.tile([P, H, N], bf16)
            nc.gpsimd.memset(Q4[:], 0.0)
            nc.gpsimd.memset(K4[:], 0.0)
            # VT[:, 4*blk + h, 0:32] = V^T[m in blk, head h dims]; [:, :, 32:64] = 1.0
            VT = main.tile([P, 64, 64], bf16)
            av_all = main.tile([P, NH], bf16)
            mh_sb = main.tile([P, NH], bf16)
            h1_sb = main.tile([P, 2, NH], bf16)
            out_sb = main.tile([P, NH], f32)

            # ---- projections ----
            with (
                tc.tile_pool(name="ppsum", bufs=2, space="PSUM") as pp,
                tc.tile_pool(name="vtpsum", bufs=2, space="PSUM") as vp,
            ):
                for c in range(2):
                    q_ps = pp.tile([P, 512], f32, tag="qk")
                    nc.tensor.matmul(
                        q_ps[:], wq_t[:], x1r[:, ts(c, 512)], start=True, stop=True
                    )
                    for h in range(H):
                        # shifted single-src op: srcs at rows 32h..32h+32,
                        # out at rows 0..32 (HW-verified pattern)
                        nc.vector.tensor_scalar_add(
                            Q4[0:32, h, ts(c, 512)],
                            q_ps[32 * h : 32 * h + 32, :],
                            bq_t[32 * h : 32 * h + 32, :],
                        )
                for c in range(4):
                    k_ps = pp.tile([P, 512], f32, tag="qk")
                    nc.tensor.matmul(
                        k_ps[:], wk_t[:], x2r[:, ts(c, 512)], start=True, stop=True
                    )
                    for h in range(H):
                        nc.vector.tensor_copy(
                            K4[0:32, h, ts(c, 512)], k_ps[32 * h : 32 * h + 32, :]
                        )

                # ones columns: VT[:, :, 32:64] = 1.0 (computed as x2*0 + 1 on DVE;
                # memset can't target strided non-fp32 APs)
                nc.vector.tensor_scalar(
                    VT[:, :, 32:64],
                    x2stg.rearrange("p (a b) -> p a b", a=64),
                    0.0,
                    1.0,
                    OP.mult,
                    OP.add,
                )
                for blk in range(16):
                    vt_ps = vp.tile([P, P], f32, tag="vt")
                    nc.tensor.matmul(
                        vt_ps[:], x2r[:, ts(blk, 128)], wv_t[:], start=True, stop=True
                    )
                    nc.vector.tensor_copy(
                        VT[:, 4 * blk : 4 * blk + 4, 0:32],
                        vt_ps.rearrange("p (h d) -> p h d", h=4),
                    )

            # ---- attention ----
            with (
                tc.tile_pool(name="spsum", bufs=2, space="PSUM") as sp,
                tc.tile_pool(name="avpsum", bufs=1, space="PSUM") as avp,
            ):
                for c in range(2):
                    # bank h (free cols h*512..) accumulates head h; rows 0-31
                    # data, rows 32-63 sum(exp) replicated.
                    av_acc = avp.tile([P, 2048], f32, tag="av")
                    for j in range(16):
                        for p in range(2):
                            st = sp.tile([P, 1024], f32, tag="st")
                            for i in range(2):
                                h = 2 * p + i
                                nc.tensor.matmul(
                                    st[:, ts(i, 512)],
                                    K4[:, h, ts(j, 128)],
                                    Q4[:, h, ts(c, 512)],
                                    start=True,
                                    stop=True,
                                )
                            et = work.tile([P, 1024], bf16, tag="exp")
                            nc.scalar.activation(et[:], st[:], AF.Exp)
                            for i in range(2):
                                h = 2 * p + i
                                nc.tensor.matmul(
                                    av_acc[0:64, ts(h, 512)],
                                    VT[:, 4 * j + h, :],
                                    et[:, ts(i, 512)],
                                    start=(j == 0),
                                    stop=(j == 15),
                                )
                    # normalize: av_all[h] = av_raw[h] / sum_exp[h]
                    for h in range(4):
                        rec = recp.tile([P, 512], f32, tag="rec")
                        nc.vector.reciprocal(
                            rec[0:32, :],
                            av_acc[32:64, ts(h, 512)],
                        )
                        nc.vector.tensor_mul(
                            av_all[32 * h : 32 * h + 32, ts(c, 512)],
                            av_acc[0:32, ts(h, 512)],
                            rec[0:32, :],
                        )

            # ---- tail: mh, concat->c1->BN(folded)->relu, c2, residual ----
            with tc.tile_pool(name="tpsum", bufs=2, space="PSUM") as tp:
                for c in range(2):
                    m_ps = tp.tile([P, 512], f32, tag="t")
                    nc.tensor.matmul(
                        m_ps[:], wm_t[:], av_all[:, ts(c, 512)], start=True, stop=True
                    )
                    nc.vector.tensor_scalar_add(mh_sb[:, ts(c, 512)], m_ps[:], bm_t[:])
                for oh in range(2):
                    for c in range(2):
                        c_ps = tp.tile([P, 512], f32, tag="t")
                        nc.tensor.matmul(
                            c_ps[:],
                            wc1_l(0, oh),
                            x1r[:, ts(c, 512)],
                            start=True,
                            stop=False,
                        )
                        nc.tensor.matmul(
                            c_ps[:],
                            wc1_l(1, oh),
                            mh_sb[:, ts(c, 512)],
                            start=False,
                            stop=True,
                        )
                        # relu(psum + b1[oh])
                        nc.vector.tensor_scalar(
                            h1_sb[:, oh, ts(c, 512)],
                            c_ps[:],
                            b1_t[:, oh : oh + 1],
                            0.0,
                            OP.add,
                            OP.max,
                        )
                for c in range(2):
                    o_ps = tp.tile([P, 512], f32, tag="t")
                    for oh in range(2):
                        nc.tensor.matmul(
                            o_ps[:],
                            wc2_l(oh),
                            h1_sb[:, oh, ts(c, 512)],
                            start=(oh == 0),
                            stop=(oh == 1),
                        )
                    # (psum + bc2) + x1s
                    nc.vector.scalar_tensor_tensor(
                        out_sb[:, ts(c, 512)],
                        o_ps[:],
                        bc2_t[:],
                        x1t[:, ts(c, 512)],
                        OP.add,
                        OP.add,
                    )
                nc.sync.dma_start(out_d[:], out_sb[:])

    nc.finalize()
    return nc


def _prep_shared(inputs):
    s = 1.0 / np.sqrt(np.float32(D))
    wq = np.asarray(inputs["wq"], np.float32)
    bq = np.asarray(inputs["bq"], np.float32)
    wk = np.asarray(inputs["wk"], np.float32)
    wv = np.asarray(inputs["wv"], np.float32)
    bv = np.asarray(inputs["bv"], np.float32)
    wm = np.asarray(inputs["wm"], np.float32)
    bm = np.asarray(inputs["bm"], np.float32)
    wc1 = np.asarray(inputs["wc1"], np.float32)
    bc1 = np.asarray(inputs["bc1"], np.float32)
    gamma = np.asarray(inputs["bn_gamma"], np.float32)
    beta = np.asarray(inputs["bn_beta"], np.float32)
    mean = np.asarray(inputs["bn_mean"], np.float32)
    var = np.asarray(inputs["bn_var"], np.float32)
    wc2 = np.asarray(inputs["wc2"], np.float32)
    bc2 = np.asarray(inputs["bc2"], np.float32)

    a = gamma / np.sqrt(var + np.float32(1e-5))
    wc1s = wc1 * a[:, None]
    b1v = (bc1 - mean) * a + beta

    def c_(x):
        return np.ascontiguousarray(x, dtype=np.float32)

    # wc1T flat layout [128, 512]: col = k*256 + o, row i = input channel k*128+i
    wc1T_flat = wc1s.T.reshape(2, P, 2 * C).transpose(1, 0, 2).reshape(P, 512)
    wc2T_flat = wc2.T.reshape(2, P, C).transpose(1, 0, 2).reshape(P, 256)
    wpack = np.concatenate(
        [wq.T * s, wk.T, wv.T, wm.T, wc1T_flat, wc2T_flat], axis=1
    )
    bpack = np.concatenate(
        [
            (bq * s).reshape(P, 1),
            np.zeros((P, 3), np.float32),
            (bm + wm @ bv).reshape(P, 1),
            b1v.reshape(2, P).T,
            bc2.reshape(P, 1),
        ],
        axis=1,
    )
    shared = {"wpack": c_(wpack), "bpack": c_(bpack)}
    return shared


def kernel(**inputs) -> np.ndarray:
    from concourse.bass_utils import run_bass_kernel_spmd

    if "nc" not in _CACHE:
        _CACHE["nc"] = _build_nc()
    nc = _CACHE["nc"]

    x1 = np.asarray(inputs["x1"], np.float32)
    x2 = np.asarray(inputs["x2"], np.float32)
    # kv_mask is all ones per the problem spec (fill=ones) -> no-op; ignored.

    shared = _prep_shared(inputs)

    core_ids = list(range(8))
    in_maps = []
    for core in core_ids:
        b, half = divmod(core, 2)
        m = dict(shared)
        m["x1s"] = np.ascontiguousarray(x1[b, :, half * NH : (half + 1) * NH])
        m["x2b"] = np.ascontiguousarray(x2[b])
        in_maps.append(m)

    res = run_bass_kernel_spmd(nc, in_maps, core_ids)
    out = np.empty((B, C, N), dtype=np.float32)
    for core in core_ids:
        b, half = divmod(core, 2)
        out[b, :, half * NH : (half + 1) * NH] = res.results[core]["out"]
    return out



# revision 7
# speedup vs baseline: 1.1939x; 1.1939x over previous
"""Trainium2 Bass kernel for nn_AttentionPropagation.

Shapes (hardcoded): B=4, C=128, H=4 heads, D=32, N=2048.
Sharding: 8 cores = (batch b) x (sequence half). Pointwise in query position n
everywhere except K/V, so each core takes x1[b,:,half] (1024 query positions)
plus the full x2[b] (keys/values), no cross-core communication.

Math folding done host-side (exact):
 - 1/sqrt(D) folded into wq/bq.
 - bk dropped: per-query score constant -> cancels in softmax.
 - bv folded into mh bias (softmax rows sum to 1).
 - BatchNorm (inference) folded into wc1/bias.
 - kv_mask is all ones per the spec -> ignored.
 - all matmul operands pre-converted to bf16 on host (no device casts).

Device kernel per core:
 - K kept in natural packed layout [C, N] as the shared stationary operand;
   per-head isolation comes from zero-padded Q4 [C, h, NH] (only rows
   32h:32h+32 nonzero), so scores_h^T = K^T @ Q4[:,h,:] per 128-key block.
 - exp split across engines: scalar ACTIVATE (exact) for even heads, DVE
   Schraudolph (x*128/ln2 + 16250.5 -> int16, bitcast bf16) for odd heads.
 - AV + softmax sums in one accumulation: VT tiles [128key, 32 V-dims | 32
   ones] per (j, h); psum rows 0:32 = raw AV, 32:64 = sum(exp) replicated.
 - normalize via DVE reciprocal_approx_fast + tensor_mul.
 - tail: mh -> concat -> c1 -> (folded BN) relu -> c2 -> + x1.
"""

import sys

import numpy as np

sys.path.insert(0, "/opt/trn_rl_repo")

_CACHE = {}

P = 128
B, C, H, D, N = 4, 128, 4, 32, 2048
NH = N // 2  # per-core query positions

SCHR_A = float(128.0 / np.log(2.0))
SCHR_B = 16250.5


def _build_nc():
    import concourse.mybir as mybir
    import concourse.tile as tile
    from concourse import bacc
    from concourse.bass import ts

    f32 = mybir.dt.float32
    bf16 = mybir.dt.bfloat16
    i16 = mybir.dt.int16
    AF = mybir.ActivationFunctionType
    OP = mybir.AluOpType

    ones_pair = float(np.frombuffer(b"\x80\x3f\x80\x3f", dtype=np.float32)[0])

    nc = bacc.Bacc()
    x1b_d = nc.declare_dram_parameter("x1b", [P, NH], bf16, isOutput=False)
    x1f_d = nc.declare_dram_parameter("x1f", [P, NH], f32, isOutput=False)
    x2b_d = nc.declare_dram_parameter("x2b", [P, N], bf16, isOutput=False)
    # weights packed bf16 (cols: wqT 0:128, wkT 128:256, wvT 256:384,
    # wmT 384:512, wc1T 512:1024 (k*256+o), wc2T 1024:1280)
    wpack = nc.declare_dram_parameter("wpack", [P, 1280], bf16, isOutput=False)
    # biases packed fp32 (cols: bq*s 0, bm' 1, b1 2:4, bc2 4)
    bpack = nc.declare_dram_parameter("bpack", [P, 8], f32, isOutput=False)
    out_d = nc.declare_dram_parameter("out", [P, NH], f32, isOutput=True)

    with tile.TileContext(nc) as tc:
        with (
            tc.tile_pool(name="consts", bufs=1) as consts,
            tc.tile_pool(name="main", bufs=1) as main,
            tc.tile_pool(name="etp", bufs=2) as etp,
            tc.tile_pool(name="recp", bufs=2) as recp,
        ):
            # ---- parallel input DMAs across engine queues ----
            wr = consts.tile([P, 1280], bf16)
            bp = consts.tile([P, 8], f32)
            x1r = main.tile([P, NH], bf16)
            x1t = main.tile([P, NH], f32)
            x2r = main.tile([P, N], bf16)
            nc.sync.dma_start(wr[:, 0:384], wpack[:, 0:384])
            nc.sync.dma_start(wr[:, 384:1280], wpack[:, 384:1280])
            nc.scalar.dma_start(x1r[:], x1b_d[:])
            nc.gpsimd.dma_start(x2r[:, 0:1024], x2b_d[:, 0:1024])
            nc.scalar.dma_start(x2r[:, 1024:2048], x2b_d[:, 1024:2048])
            nc.gpsimd.dma_start(bp[:], bpack[:])
            nc.sync.dma_start(x1t[:], x1f_d[:])

            wq_t = wr[:, 0:128]
            wk_t = wr[:, 128:256]
            wv_t = wr[:, 256:384]
            wm_t = wr[:, 384:512]

            def wc1_l(k, oh):  # lhsT chunk [128 in, 128 out]
                return wr[:, 512 + k * 256 + oh * 128 : 512 + k * 256 + oh * 128 + 128]

            def wc2_l(oh):
                return wr[:, 1024 + oh * 128 : 1024 + oh * 128 + 128]

            bq_t = bp[:, 0:1]
            bm_t = bp[:, 1:2]
            b1_t = bp[:, 2:4]
            bc2_t = bp[:, 4:5]

            # Q4[:, h, :] = head-h rows of Q at natural partitions, zeros
            # elsewhere -> per-head scores via full-128-contract matmuls
            # against natural packed K.
            Q4 = main.tile([P, H, NH], bf16)
            nc.gpsimd.memset(Q4[:].bitcast(f32), 0.0)
            # VT[:, j, h, 0:32] = V^T[key m of block j, head-h dims];
            # [:, :, :, 32:64] = 1.0 (pre-memset packed double-bf16 ones).
            VT = main.tile([P, 16, H, 64], bf16)
            nc.gpsimd.memset(VT[:].bitcast(f32), ones_pair)
            k_sb = main.tile([P, N], bf16)
            av_all = main.tile([P, NH], bf16)
            mh_sb = main.tile([P, NH], bf16)
            h1_sb = main.tile([P, 2, NH], bf16)
            out_sb = main.tile([P, NH], f32)

            # ---- projections ----
            with (
                tc.tile_pool(name="qpsum", bufs=1, space="PSUM") as qp_pool,
                tc.tile_pool(name="kpsum", bufs=2, space="PSUM") as kp_pool,
                tc.tile_pool(name="vpsum", bufs=2, space="PSUM") as vp_pool,
            ):
                q_ps = qp_pool.tile([P, NH], f32)
                for c in range(2):
                    nc.tensor.matmul(
                        q_ps[:, ts(c, 512)], wq_t[:], x1r[:, ts(c, 512)],
                        start=True, stop=True,
                    )
                # Q4[32h:32h+32, h, :] = q_ps[32h:32h+32] + bq (same base)
                for h in range(H):
                    sl = slice(32 * h, 32 * h + 32)
                    eng = nc.scalar if h % 2 == 0 else nc.vector
                    if eng is nc.scalar:
                        nc.scalar.activation(
                            Q4[sl, h, :], q_ps[sl, :], AF.Identity, bias=bq_t[sl, :]
                        )
                    else:
                        nc.vector.tensor_scalar_add(
                            Q4[sl, h, :], q_ps[sl, :], bq_t[sl, :]
                        )

                for c in range(4):
                    k_ps = kp_pool.tile([P, 512], f32, tag="kp")
                    nc.tensor.matmul(
                        k_ps[:], wk_t[:], x2r[:, ts(c, 512)], start=True, stop=True
                    )
                    nc.scalar.copy(k_sb[:, ts(c, 512)], k_ps[:])

                for g in range(4):  # groups of 4 key-blocks
                    v_ps = vp_pool.tile([P, 512], f32, tag="vp")
                    for m in range(4):
                        nc.tensor.matmul(
                            v_ps[:, ts(m, 128)],
                            x2r[:, ts(4 * g + m, 128)],
                            wv_t[:],
                            start=True, stop=True,
                        )
                    # strided scatter [p, (j h d)] -> VT[:, j, h, 32:64]
                    # (cols 0:32 stay 1.0 from the memset -> sums at av rows
                    # 0:32, base-0-aligned for reciprocal_approx_fast)
                    nc.vector.tensor_copy(
                        VT[:, 4 * g : 4 * g + 4, :, 32:64],
                        v_ps.rearrange("p (j h d) -> p j h d", j=4, h=4),
                    )

            # ---- attention: head pairs (scalar exp even head, DVE odd) ----
            with (
                tc.tile_pool(name="avpsum", bufs=1, space="PSUM") as avp,
                tc.tile_pool(name="spsum", bufs=2, space="PSUM") as sp,
            ):
                for hp in range(2):
                    h0, h1 = 2 * hp, 2 * hp + 1
                    av0 = avp.tile([P, NH], f32, tag="av0")
                    av1 = avp.tile([P, NH], f32, tag="av1")
                    for j in range(16):
                        kblk = k_sb[:, ts(j, 128)]
                        st0 = sp.tile([P, NH], f32, tag="st")
                        for c in range(2):
                            nc.tensor.matmul(
                                st0[:, ts(c, 512)], kblk, Q4[:, h0, ts(c, 512)],
                                start=True, stop=True,
                            )
                        st1 = sp.tile([P, NH], f32, tag="st")
                        for c in range(2):
                            nc.tensor.matmul(
                                st1[:, ts(c, 512)], kblk, Q4[:, h1, ts(c, 512)],
                                start=True, stop=True,
                            )
                        et0 = etp.tile([P, NH], bf16, tag="et0")
                        nc.scalar.activation(et0[:], st0[:], AF.Exp)
                        et1 = etp.tile([P, NH], i16, tag="et1")
                        nc.vector.tensor_scalar(
                            et1[:], st1[:], SCHR_A, SCHR_B, OP.mult, OP.add
                        )
                        for c in range(2):
                            nc.tensor.matmul(
                                av0[0:64, ts(c, 512)],
                                VT[:, j, h0, :],
                                et0[:, ts(c, 512)],
                                start=(j == 0), stop=(j == 15),
                            )
                        for c in range(2):
                            nc.tensor.matmul(
                                av1[0:64, ts(c, 512)],
                                VT[:, j, h1, :],
                                et1[:, ts(c, 512)].bitcast(bf16),
                                start=(j == 0), stop=(j == 15),
                            )
                    for i, av in ((0, av0), (1, av1)):
                        h = 2 * hp + i
                        rec = recp.tile([P, NH], f32, tag="rec")
                        nc.vector.reciprocal_approx_fast(
                            out=rec[0:32, :], in_=av[0:32, :]
                        )
                        nc.vector.tensor_mul(
                            av_all[32 * h : 32 * h + 32, :], av[32:64, :], rec[0:32, :]
                        )

            # ---- tail: mh, concat->c1->(folded BN) relu, c2, residual ----
            with tc.tile_pool(name="tpsum", bufs=2, space="PSUM") as tp:
                m_ps = tp.tile([P, NH], f32, tag="mps", bufs=1)
                for c in range(2):
                    nc.tensor.matmul(
                        m_ps[:, ts(c, 512)], wm_t[:], av_all[:, ts(c, 512)],
                        start=True, stop=True,
                    )
                nc.scalar.activation(mh_sb[:], m_ps[:], AF.Identity, bias=bm_t[:])
                for oh in range(2):
                    c_ps = tp.tile([P, NH], f32, tag="cps")
                    for c in range(2):
                        nc.tensor.matmul(
                            c_ps[:, ts(c, 512)], wc1_l(0, oh), x1r[:, ts(c, 512)],
                            start=True, stop=False,
                        )
                        nc.tensor.matmul(
                            c_ps[:, ts(c, 512)], wc1_l(1, oh), mh_sb[:, ts(c, 512)],
                            start=False, stop=True,
                        )
                    # relu(psum + b1[oh])
                    if oh == 0:
                        nc.vector.tensor_scalar(
                            h1_sb[:, oh, :], c_ps[:], b1_t[:, oh : oh + 1], 0.0,
                            OP.add, OP.max,
                        )
                    else:
                        nc.scalar.activation(
                            h1_sb[:, oh, :], c_ps[:], AF.Relu,
                            bias=b1_t[:, oh : oh + 1],
                        )
                o_ps = tp.tile([P, NH], f32, tag="ops", bufs=1)
                for c in range(2):
                    for oh in range(2):
                        nc.tensor.matmul(
                            o_ps[:, ts(c, 512)], wc2_l(oh), h1_sb[:, oh, ts(c, 512)],
                            start=(oh == 0), stop=(oh == 1),
                        )
                    # (psum + bc2) + x1
                    nc.vector.scalar_tensor_tensor(
                        out_sb[:, ts(c, 512)], o_ps[:, ts(c, 512)], bc2_t[:],
                        x1t[:, ts(c, 512)], OP.add, OP.add,
                    )
                    nc.sync.dma_start(out_d[:, ts(c, 512)], out_sb[:, ts(c, 512)])

    nc.finalize()
    return nc


def _prep_shared(inputs):
    import ml_dtypes

    bf = ml_dtypes.bfloat16
    s = 1.0 / np.sqrt(np.float32(D))
    wq = np.asarray(inputs["wq"], np.float32)
    bq = np.asarray(inputs["bq"], np.float32)
    wk = np.asarray(inputs["wk"], np.float32)
    wv = np.asarray(inputs["wv"], np.float32)
    bv = np.asarray(inputs["bv"], np.float32)
    wm = np.asarray(inputs["wm"], np.float32)
    bm = np.asarray(inputs["bm"], np.float32)
    wc1 = np.asarray(inputs["wc1"], np.float32)
    bc1 = np.asarray(inputs["bc1"], np.float32)
    gamma = np.asarray(inputs["bn_gamma"], np.float32)
    beta = np.asarray(inputs["bn_beta"], np.float32)
    mean = np.asarray(inputs["bn_mean"], np.float32)
    var = np.asarray(inputs["bn_var"], np.float32)
    wc2 = np.asarray(inputs["wc2"], np.float32)
    bc2 = np.asarray(inputs["bc2"], np.float32)

    a = gamma / np.sqrt(var + np.float32(1e-5))
    wc1s = wc1 * a[:, None]
    b1v = (bc1 - mean) * a + beta

    # wc1T flat layout [128, 512]: col = k*256 + o; wc2T flat [128, 256]
    wc1T_flat = wc1s.T.reshape(2, P, 2 * C).transpose(1, 0, 2).reshape(P, 512)
    wc2T_flat = wc2.T.reshape(2, P, C).transpose(1, 0, 2).reshape(P, 256)
    wpack = np.concatenate(
        [wq.T * s, wk.T, wv.T, wm.T, wc1T_flat, wc2T_flat], axis=1
    )
    bpack = np.concatenate(
        [
            (bq * s).reshape(P, 1),
            (bm + wm @ bv).reshape(P, 1),
            b1v.reshape(2, P).T,
            bc2.reshape(P, 1),
            np.zeros((P, 3), np.float32),
        ],
        axis=1,
    )
    return {
        "wpack": np.ascontiguousarray(wpack.astype(bf)),
        "bpack": np.ascontiguousarray(bpack, dtype=np.float32),
    }


def kernel(**inputs) -> np.ndarray:
    import ml_dtypes

    from concourse.bass_utils import run_bass_kernel_spmd

    bf = ml_dtypes.bfloat16
    if "nc" not in _CACHE:
        _CACHE["nc"] = _build_nc()
    nc = _CACHE["nc"]

    x1 = np.asarray(inputs["x1"], np.float32)
    x2 = np.asarray(inputs["x2"], np.float32)
    # kv_mask is all ones per the problem spec -> no-op; ignored.

    shared = _prep_shared(inputs)

    core_ids = list(range(8))
    in_maps = []
    for core in core_ids:
        b, half = divmod(core, 2)
        m = dict(shared)
        x1s = x1[b, :, half * NH : (half + 1) * NH]
        m["x1b"] = np.ascontiguousarray(x1s.astype(bf))
        m["x1f"] = np.ascontiguousarray(x1s)
        m["x2b"] = np.ascontiguousarray(x2[b].astype(bf))
        in_maps.append(m)

    res = run_bass_kernel_spmd(nc, in_maps, core_ids)
    out = np.empty((B, C, N), dtype=np.float32)
    for core in core_ids:
        b, half = divmod(core, 2)
        out[b, :, half * NH : (half + 1) * NH] = res.results[core]["out"]
    return out


# revision 9
# speedup vs baseline: 1.4840x; 1.2430x over previous
"""Trainium2 Bass kernel for nn_AttentionPropagation.

Shapes (hardcoded): B=4, C=128, H=4 heads, D=32, N=2048.
Sharding: 8 cores = (batch b) x (sequence half). Pointwise in query position n
everywhere except K/V, so each core takes x1[b,:,half] (1024 query positions)
plus the full x2[b] (keys/values), no cross-core communication.

Math folding done host-side (exact):
 - 1/sqrt(D) folded into wq/bq.
 - bk dropped: per-query score constant -> cancels in softmax.
 - bv folded into mh bias (softmax rows sum to 1).
 - BatchNorm (inference) folded into wc1/bias.
 - kv_mask is all ones per the spec -> ignored.
 - all matmul operands pre-converted to bf16 on host (no device casts).

Device kernel per core:
 - K kept in natural packed layout [C, N] as the shared stationary operand;
   per-head isolation comes from zero-padded Q4 [C, h, NH] (only rows
   32h:32h+32 nonzero), so scores_h^T = K^T @ Q4[:,h,:] per 128-key block.
 - exp split across engines: scalar ACTIVATE (exact) for even heads, DVE
   Schraudolph (x*128/ln2 + 16250.5 -> int16, bitcast bf16) for odd heads.
 - AV + softmax sums in one accumulation: VT tiles [128key, 32 V-dims | 32
   ones] per (j, h); psum rows 0:32 = raw AV, 32:64 = sum(exp) replicated.
 - normalize via DVE reciprocal_approx_fast + tensor_mul.
 - tail: mh -> concat -> c1 -> (folded BN) relu -> c2 -> + x1.
"""

import sys

import numpy as np

sys.path.insert(0, "/opt/trn_rl_repo")

_CACHE = {}

P = 128
B, C, H, D, N = 4, 128, 4, 32, 2048
NH = N // 2  # per-core query positions

SCHR_A = float(128.0 / np.log(2.0))
SCHR_B = 16250.5


def _build_nc():
    import concourse.mybir as mybir
    import concourse.tile as tile
    from concourse import bacc
    from concourse.bass import ts

    f32 = mybir.dt.float32
    bf16 = mybir.dt.bfloat16
    i16 = mybir.dt.int16
    AF = mybir.ActivationFunctionType
    OP = mybir.AluOpType

    ones_pair = float(np.frombuffer(b"\x80\x3f\x80\x3f", dtype=np.float32)[0])

    nc = bacc.Bacc()
    x1b_d = nc.declare_dram_parameter("x1b", [P, NH], bf16, isOutput=False)
    x1f_d = nc.declare_dram_parameter("x1f", [P, NH], f32, isOutput=False)
    x2b_d = nc.declare_dram_parameter("x2b", [P, N], bf16, isOutput=False)
    # weights packed bf16 (cols: wqT 0:128, wkT 128:256, wvT 256:384,
    # wmT 384:512, wc1T 512:1024 (k*256+o), wc2T 1024:1280)
    wpack = nc.declare_dram_parameter("wpack", [P, 1280], bf16, isOutput=False)
    # biases packed fp32 (cols: bq*s 0, bm' 1, b1 2:4, bc2 4)
    bpack = nc.declare_dram_parameter("bpack", [P, 8], f32, isOutput=False)
    out_d = nc.declare_dram_parameter("out", [P, NH], f32, isOutput=True)

    with tile.TileContext(nc) as tc:
        with (
            tc.tile_pool(name="consts", bufs=1) as consts,
            tc.tile_pool(name="main", bufs=1) as main,
            tc.tile_pool(name="etp", bufs=4) as etp,
            tc.tile_pool(name="recp", bufs=2) as recp,
        ):
            # ---- parallel input DMAs across engine queues ----
            wr = consts.tile([P, 1280], bf16)
            bp = consts.tile([P, 8], f32)
            x1r = main.tile([P, NH], bf16)
            x1t = main.tile([P, NH], f32)
            x2r = main.tile([P, N], bf16)
            nc.sync.dma_start(wr[:, 0:384], wpack[:, 0:384])
            nc.sync.dma_start(wr[:, 384:1280], wpack[:, 384:1280])
            nc.scalar.dma_start(x1r[:], x1b_d[:])
            nc.gpsimd.dma_start(x2r[:, 0:1024], x2b_d[:, 0:1024])
            nc.scalar.dma_start(x2r[:, 1024:2048], x2b_d[:, 1024:2048])
            nc.gpsimd.dma_start(bp[:], bpack[:])
            nc.sync.dma_start(x1t[:], x1f_d[:])

            wq_t = wr[:, 0:128]
            wk_t = wr[:, 128:256]
            wv_t = wr[:, 256:384]
            wm_t = wr[:, 384:512]

            def wc1_l(k, oh):  # lhsT chunk [128 in, 128 out]
                return wr[:, 512 + k * 256 + oh * 128 : 512 + k * 256 + oh * 128 + 128]

            def wc2_l(oh):
                return wr[:, 1024 + oh * 128 : 1024 + oh * 128 + 128]

            bq_t = bp[:, 0:1]
            bm_t = bp[:, 1:2]
            b1_t = bp[:, 2:4]
            bc2_t = bp[:, 4:5]

            # Q4[:, h, :] = head-h rows of Q at natural partitions, zeros
            # elsewhere -> per-head scores via full-128-contract matmuls
            # against natural packed K.
            Q4 = main.tile([P, H, NH], bf16)
            nc.gpsimd.memset(Q4[:].bitcast(f32), 0.0)
            # VT[:, j, h, 0:32] = V^T[key m of block j, head-h dims];
            # [:, :, :, 32:64] = 1.0 (pre-memset packed double-bf16 ones).
            VT = main.tile([P, 16, H, 64], bf16)
            nc.gpsimd.memset(VT[:].bitcast(f32), ones_pair)
            k_sb = main.tile([P, N], bf16)
            av_all = main.tile([P, NH], bf16)
            mh_sb = main.tile([P, NH], bf16)
            h1_sb = main.tile([P, 2, NH], bf16)
            out_sb = main.tile([P, NH], f32)

            # ---- projections ----
            with (
                tc.tile_pool(name="qpsum", bufs=1, space="PSUM") as qp_pool,
                tc.tile_pool(name="kpsum", bufs=2, space="PSUM") as kp_pool,
                tc.tile_pool(name="vpsum", bufs=2, space="PSUM") as vp_pool,
            ):
                q_ps = qp_pool.tile([P, NH], f32)
                for c in range(2):
                    nc.tensor.matmul(
                        q_ps[:, ts(c, 512)], wq_t[:], x1r[:, ts(c, 512)],
                        start=True, stop=True,
                    )
                # Q4[32h:32h+32, h, :] = q_ps[32h:32h+32] + bq (same base)
                for h in range(H):
                    sl = slice(32 * h, 32 * h + 32)
                    eng = nc.scalar if h % 2 == 0 else nc.vector
                    if eng is nc.scalar:
                        nc.scalar.activation(
                            Q4[sl, h, :], q_ps[sl, :], AF.Identity, bias=bq_t[sl, :]
                        )
                    else:
                        nc.vector.tensor_scalar_add(
                            Q4[sl, h, :], q_ps[sl, :], bq_t[sl, :]
                        )

                for c in range(4):
                    k_ps = kp_pool.tile([P, 512], f32, tag="kp")
                    nc.tensor.matmul(
                        k_ps[:], wk_t[:], x2r[:, ts(c, 512)], start=True, stop=True
                    )
                    nc.scalar.copy(k_sb[:, ts(c, 512)], k_ps[:])

                for g in range(4):  # groups of 4 key-blocks
                    v_ps = vp_pool.tile([P, 512], f32, tag="vp")
                    for m in range(4):
                        nc.tensor.matmul(
                            v_ps[:, ts(m, 128)],
                            x2r[:, ts(4 * g + m, 128)],
                            wv_t[:],
                            start=True, stop=True,
                        )
                    # strided scatter [p, (j h d)] -> VT[:, j, h, 32:64]
                    # (cols 0:32 stay 1.0 from the memset -> sums at av rows
                    # 0:32, base-0-aligned for reciprocal_approx_fast)
                    nc.vector.tensor_copy(
                        VT[:, 4 * g : 4 * g + 4, :, 32:64],
                        v_ps.rearrange("p (j h d) -> p j h d", j=4, h=4),
                    )

            # ---- attention: head pairs (scalar exp even head, DVE odd) ----
            with (
                tc.tile_pool(name="avpsum", bufs=1, space="PSUM") as avp,
                tc.tile_pool(name="spsum", bufs=4, space="PSUM") as sp,
            ):
                for hp in range(2):
                    h0, h1 = 2 * hp, 2 * hp + 1
                    av0 = avp.tile([P, NH], f32, tag="av0")
                    av1 = avp.tile([P, NH], f32, tag="av1")
                    for j in range(16):
                        kblk = k_sb[:, ts(j, 128)]
                        # 4 single-bank score tiles per j: deep pipeline so
                        # exp results are ready before the PE needs them.
                        sts = []
                        for q, (h, c) in enumerate(
                            ((h0, 0), (h0, 1), (h1, 0), (h1, 1))
                        ):
                            st = sp.tile([P, 512], f32, tag="st", name=f"st{q}")
                            nc.tensor.matmul(
                                st[:], kblk, Q4[:, h, ts(c, 512)],
                                start=True, stop=True,
                            )
                            sts.append(st)
                        # exp: scalar engine (exact) on 2-3 tiles, DVE
                        # Schraudolph on the rest (alternating split 2.5/1.5)
                        n_scalar = 3 if j % 2 == 0 else 2
                        ets = []
                        for q in range(4):
                            if q < n_scalar:
                                et = etp.tile([P, 512], bf16, tag="etb",
                                              name=f"et{q}")
                                nc.scalar.activation(et[:], sts[q][:], AF.Exp)
                            else:
                                eti = etp.tile([P, 512], i16, tag="eti",
                                               name=f"et{q}")
                                nc.vector.tensor_scalar(
                                    eti[:], sts[q][:], SCHR_A, SCHR_B,
                                    OP.mult, OP.add,
                                )
                                et = eti.bitcast(bf16)
                            ets.append(et)
                        for q, (av, h, c) in enumerate(
                            ((av0, h0, 0), (av0, h0, 1), (av1, h1, 0),
                             (av1, h1, 1))
                        ):
                            nc.tensor.matmul(
                                av[0:64, ts(c, 512)],
                                VT[:, j, h, :],
                                ets[q][:],
                                start=(j == 0), stop=(j == 15),
                            )
                    for i, av in ((0, av0), (1, av1)):
                        h = 2 * hp + i
                        rec = recp.tile([P, NH], f32, tag="rec")
                        nc.vector.reciprocal_approx_fast(
                            out=rec[0:32, :], in_=av[0:32, :]
                        )
                        nc.vector.tensor_mul(
                            av_all[32 * h : 32 * h + 32, :], av[32:64, :], rec[0:32, :]
                        )

            # ---- tail: mh, concat->c1->(folded BN) relu, c2, residual ----
            with tc.tile_pool(name="tpsum", bufs=2, space="PSUM") as tp:
                m_ps = tp.tile([P, NH], f32, tag="mps", bufs=1)
                for c in range(2):
                    nc.tensor.matmul(
                        m_ps[:, ts(c, 512)], wm_t[:], av_all[:, ts(c, 512)],
                        start=True, stop=True,
                    )
                nc.scalar.activation(mh_sb[:], m_ps[:], AF.Identity, bias=bm_t[:])
                for oh in range(2):
                    c_ps = tp.tile([P, NH], f32, tag="cps")
                    for c in range(2):
                        nc.tensor.matmul(
                            c_ps[:, ts(c, 512)], wc1_l(0, oh), x1r[:, ts(c, 512)],
                            start=True, stop=False,
                        )
                        nc.tensor.matmul(
                            c_ps[:, ts(c, 512)], wc1_l(1, oh), mh_sb[:, ts(c, 512)],
                            start=False, stop=True,
                        )
                    # relu(psum + b1[oh])
                    if oh == 0:
                        nc.vector.tensor_scalar(
                            h1_sb[:, oh, :], c_ps[:], b1_t[:, oh : oh + 1], 0.0,
                            OP.add, OP.max,
                        )
                    else:
                        nc.scalar.activation(
                            h1_sb[:, oh, :], c_ps[:], AF.Relu,
                            bias=b1_t[:, oh : oh + 1],
                        )
                o_ps = tp.tile([P, NH], f32, tag="ops", bufs=1)
                for c in range(2):
                    for oh in range(2):
                        nc.tensor.matmul(
                            o_ps[:, ts(c, 512)], wc2_l(oh), h1_sb[:, oh, ts(c, 512)],
                            start=(oh == 0), stop=(oh == 1),
                        )
                    # (psum + bc2) + x1
                    nc.vector.scalar_tensor_tensor(
                        out_sb[:, ts(c, 512)], o_ps[:, ts(c, 512)], bc2_t[:],
                        x1t[:, ts(c, 512)], OP.add, OP.add,
                    )
                    nc.sync.dma_start(out_d[:, ts(c, 512)], out_sb[:, ts(c, 512)])

    nc.finalize()
    return nc


def _prep_shared(inputs):
    import ml_dtypes

    bf = ml_dtypes.bfloat16
    s = 1.0 / np.sqrt(np.float32(D))
    wq = np.asarray(inputs["wq"], np.float32)
    bq = np.asarray(inputs["bq"], np.float32)
    wk = np.asarray(inputs["wk"], np.float32)
    wv = np.asarray(inputs["wv"], np.float32)
    bv = np.asarray(inputs["bv"], np.float32)
    wm = np.asarray(inputs["wm"], np.float32)
    bm = np.asarray(inputs["bm"], np.float32)
    wc1 = np.asarray(inputs["wc1"], np.float32)
    bc1 = np.asarray(inputs["bc1"], np.float32)
    gamma = np.asarray(inputs["bn_gamma"], np.float32)
    beta = np.asarray(inputs["bn_beta"], np.float32)
    mean = np.asarray(inputs["bn_mean"], np.float32)
    var = np.asarray(inputs["bn_var"], np.float32)
    wc2 = np.asarray(inputs["wc2"], np.float32)
    bc2 = np.asarray(inputs["bc2"], np.float32)

    a = gamma / np.sqrt(var + np.float32(1e-5))
    wc1s = wc1 * a[:, None]
    b1v = (bc1 - mean) * a + beta

    # wc1T flat layout [128, 512]: col = k*256 + o; wc2T flat [128, 256]
    wc1T_flat = wc1s.T.reshape(2, P, 2 * C).transpose(1, 0, 2).reshape(P, 512)
    wc2T_flat = wc2.T.reshape(2, P, C).transpose(1, 0, 2).reshape(P, 256)
    wpack = np.concatenate(
        [wq.T * s, wk.T, wv.T, wm.T, wc1T_flat, wc2T_flat], axis=1
    )
    bpack = np.concatenate(
        [
            (bq * s).reshape(P, 1),
            (bm + wm @ bv).reshape(P, 1),
            b1v.reshape(2, P).T,
            bc2.reshape(P, 1),
            np.zeros((P, 3), np.float32),
        ],
        axis=1,
    )
    return {
        "wpack": np.ascontiguousarray(wpack.astype(bf)),
        "bpack": np.ascontiguousarray(bpack, dtype=np.float32),
    }


def kernel(**inputs) -> np.ndarray:
    import ml_dtypes

    from concourse.bass_utils import run_bass_kernel_spmd

    bf = ml_dtypes.bfloat16
    if "nc" not in _CACHE:
        _CACHE["nc"] = _build_nc()
    nc = _CACHE["nc"]

    x1 = np.asarray(inputs["x1"], np.float32)
    x2 = np.asarray(inputs["x2"], np.float32)
    # kv_mask is all ones per the problem spec -> no-op; ignored.

    shared = _prep_shared(inputs)

    core_ids = list(range(8))
    in_maps = []
    for core in core_ids:
        b, half = divmod(core, 2)
        m = dict(shared)
        x1s = x1[b, :, half * NH : (half + 1) * NH]
        m["x1b"] = np.ascontiguousarray(x1s.astype(bf))
        m["x1f"] = np.ascontiguousarray(x1s)
        m["x2b"] = np.ascontiguousarray(x2[b].astype(bf))
        in_maps.append(m)

    res = run_bass_kernel_spmd(nc, in_maps, core_ids)
    out = np.empty((B, C, N), dtype=np.float32)
    for core in core_ids:
        b, half = divmod(core, 2)
        out[b, :, half * NH : (half + 1) * NH] = res.results[core]["out"]
    return out
